# revision 7
# baseline (speedup 1.0000x reference)
"""Trainium2 Bass kernel for nn_EnhancedTFNLayer (RBF field projection +
head interference + diffusion + gated MLP + diffusion + resampling + LN/proj/LN).

Data-parallel over batch: B=16 split as 2 batches per core across 8 cores.
All heavy math runs on device; small parameter matrices are folded on host:
  - stage-2 head mixing:  Wint_eff = I + kron(alpha.T, I_DH) @ w_int
  - diffusion (uniform per-channel coef): left-multiply by A3 = (I + dt*c*L)^3
  - RBF row scale G folded into AG = A3_1 @ diag(G)
  - sampling + 2nd diffusion: SA = S @ A3_2  (hat-function interp matrix S)
  - LN1 gamma/beta + both residuals around Wout: Wfinal = diag(g1) @ (Wout + I)
"""
import sys

sys.path.insert(0, "/opt/trn_rl_repo")

import numpy as np
import ml_dtypes

import concourse.bass as bass
import concourse.tile as tile
from concourse import mybir, bacc
from concourse.bass_utils import run_bass_kernel_spmd

B, N, D, M, H = 16, 2048, 512, 512, 8
DH = D // H
PDE_STEPS = 3
DT = 0.01
NCORES = 8
BPC = B // NCORES  # batches per core
NCH = N // 128     # 16 token chunks
DCH = D // 128     # 4
MCH = M // 128     # 4

F32 = mybir.dt.float32
BF16 = mybir.dt.bfloat16
AF = mybir.ActivationFunctionType
ALU = mybir.AluOpType


def _laplacian(m):
    L = np.zeros((m, m), np.float64)
    for i in range(m):
        L[i, i] -= 2.0
        L[i, max(i - 1, 0)] += 1.0
        L[i, min(i + 1, m - 1)] += 1.0
    return L


def _build_program(cfg):
    """cfg: dict of identity-flags controlling optional passes."""
    nc = bacc.Bacc("TRN2", target_bir_lowering=False, debug=False,
                   enable_asserts=False, num_devices=NCORES)

    # ---- external inputs (per core) ----
    emb_d = nc.dram_tensor("emb", (BPC, NCH, 128, D), BF16, kind="ExternalInput")
    psc_d = nc.dram_tensor("psc", (BPC, 128, NCH), F32, kind="ExternalInput")
    pbi_d = nc.dram_tensor("pbi", (BPC, 128, NCH), F32, kind="ExternalInput")
    grid_d = nc.dram_tensor("gridb", (128, M), F32, kind="ExternalInput")
    agt_d = nc.dram_tensor("agt", (MCH, 128, M), BF16, kind="ExternalInput")
    wint_d = nc.dram_tensor("wint", (DCH, 128, D), BF16, kind="ExternalInput")
    w1_d = nc.dram_tensor("w1", (DCH, 128, D), BF16, kind="ExternalInput")
    w2_d = nc.dram_tensor("w2", (DCH, 128, D), BF16, kind="ExternalInput")
    wfin_d = nc.dram_tensor("wfin", (DCH, 128, D), BF16, kind="ExternalInput")
    sat_d = nc.dram_tensor("sat", (BPC, MCH, 128, N), BF16, kind="ExternalInput")
    bint_d = nc.dram_tensor("bint", (128, DCH), F32, kind="ExternalInput")
    b1_d = nc.dram_tensor("b1c", (128, DCH), F32, kind="ExternalInput")
    b2_d = nc.dram_tensor("b2c", (128, DCH), F32, kind="ExternalInput")
    bfin_d = nc.dram_tensor("bfinb", (128, D), F32, kind="ExternalInput")
    g2_d = nc.dram_tensor("g2b", (128, D), F32, kind="ExternalInput")
    b2b_d = nc.dram_tensor("b2b", (128, D), F32, kind="ExternalInput")

    out_d = nc.dram_tensor("out", (BPC, NCH, 128, D), F32, kind="ExternalOutput")


    from contextlib import ExitStack
    with tile.TileContext(nc) as tc, ExitStack() as ctx:
        consts = ctx.enter_context(tc.tile_pool(name="consts", bufs=1))
        ep_pool = ctx.enter_context(tc.tile_pool(name="ep", bufs=4))
        t1_pool = ctx.enter_context(tc.tile_pool(name="t1p", bufs=MCH + 1))
        q1_pool = ctx.enter_context(tc.tile_pool(name="q1p", bufs=DCH + 1))
        f3t_pool = ctx.enter_context(tc.tile_pool(name="f3p", bufs=DCH + 1))
        gt_pool = ctx.enter_context(tc.tile_pool(name="gtp", bufs=DCH + 1))
        f4t_pool = ctx.enter_context(tc.tile_pool(name="f4p", bufs=DCH + 1))
        f4n_pool = ctx.enter_context(tc.tile_pool(name="f4n", bufs=BPC * MCH + 1))
        dram_pool = ctx.enter_context(tc.tile_pool(name="dram", bufs=2, space="DRAM"))
        x_pool = ctx.enter_context(tc.tile_pool(name="xp", bufs=NCH + 2))
        y_pool = ctx.enter_context(tc.tile_pool(name="yp", bufs=NCH + 2))
        z_pool = ctx.enter_context(tc.tile_pool(name="zp", bufs=3))
        zt_pool = ctx.enter_context(tc.tile_pool(name="ztp", bufs=DCH + 1))
        scr_pool = ctx.enter_context(tc.tile_pool(name="scr", bufs=2))
        stats = ctx.enter_context(tc.tile_pool(name="st", bufs=2))
        psA = ctx.enter_context(tc.tile_pool(name="psA", bufs=MCH, space="PSUM"))
        psB = ctx.enter_context(tc.tile_pool(name="psB", bufs=2, space="PSUM"))
        psC = ctx.enter_context(tc.tile_pool(name="psC", bufs=2, space="PSUM"))
        if True:
            # ---- load constants ----
            grid_b = consts.tile([128, M], F32)
            nc.sync.dma_start(out=grid_b, in_=grid_d[:, :])
            psc = consts.tile([128, BPC, NCH], F32)
            nc.sync.dma_start(out=psc, in_=psc_d.rearrange("b p c -> p b c"))
            pbi = consts.tile([128, BPC, NCH], F32)
            nc.sync.dma_start(out=pbi, in_=pbi_d.rearrange("b p c -> p b c"))
            agt = consts.tile([128, MCH, M], BF16)
            nc.sync.dma_start(out=agt, in_=agt_d.rearrange("c p m -> p c m"))
            wint = consts.tile([128, DCH, D], BF16)
            nc.sync.dma_start(out=wint, in_=wint_d.rearrange("c p d -> p c d"))
            w1 = consts.tile([128, DCH, D], BF16)
            nc.sync.dma_start(out=w1, in_=w1_d.rearrange("c p d -> p c d"))
            w2 = consts.tile([128, DCH, D], BF16)
            nc.sync.dma_start(out=w2, in_=w2_d.rearrange("c p d -> p c d"))
            wfin = consts.tile([128, DCH, D], BF16)
            nc.sync.dma_start(out=wfin, in_=wfin_d.rearrange("c p d -> p c d"))
            bint = consts.tile([128, DCH], F32)
            nc.sync.dma_start(out=bint, in_=bint_d[:, :])
            b1c = consts.tile([128, DCH], F32)
            nc.sync.dma_start(out=b1c, in_=b1_d[:, :])
            b2c = consts.tile([128, DCH], F32)
            nc.sync.dma_start(out=b2c, in_=b2_d[:, :])
            if not cfg["bfin_zero"]:
                bfinb = consts.tile([128, D], F32)
                nc.sync.dma_start(out=bfinb, in_=bfin_d[:, :])
            if not cfg["ln2_identity"]:
                g2b = consts.tile([128, D], F32)
                nc.sync.dma_start(out=g2b, in_=g2_d[:, :])
                b2b = consts.tile([128, D], F32)
                nc.sync.dma_start(out=b2b, in_=b2b_d[:, :])
            eps_t = consts.tile([128, 1], F32)
            nc.vector.memset(eps_t, 1e-5)

            emb_s = consts.tile([128, BPC, NCH, D], BF16)
            for b in range(BPC):
                nc.sync.dma_start(out=emb_s[:, b],
                                  in_=emb_d[b].rearrange("c p d -> p c d"))
            sat_s = consts.tile([128, BPC, MCH, N], BF16)
            for b in range(BPC):
                nc.sync.dma_start(out=sat_s[:, b],
                                  in_=sat_d[b].rearrange("c p n -> p c n"))

            # per-batch persistent intermediates
            f3t = [[None] * DCH for _ in range(BPC)]
            f4n = [[None] * MCH for _ in range(BPC)]
            f4t_dram = [None] * BPC

            # ================= phase A: EP + RBF + AG + Wint + W1 + W2 ========
            for b in range(BPC):
                # --- EP build + RBF matmul (t1[m,d] = sum_n EP[n,m] emb[n,d]) ---
                t1_ps = [psA.tile([128, D], F32, name="t1ps") for _ in range(MCH)]
                for i in range(NCH):
                    ep = ep_pool.tile([128, M], BF16, name="ep")
                    nc.scalar.activation(out=ep, in_=grid_b, func=AF.Exp,
                                         scale=psc[:, b, i:i + 1],
                                         bias=pbi[:, b, i:i + 1])
                    for j in range(MCH):
                        nc.tensor.matmul(t1_ps[j], ep[:, j * 128:(j + 1) * 128],
                                         emb_s[:, b, i, :],
                                         start=(i == 0), stop=(i == NCH - 1))
                t1 = []
                for j in range(MCH):
                    t = t1_pool.tile([128, D], BF16, name="t1")
                    nc.vector.tensor_copy(t, t1_ps[j])
                    t1.append(t)

                # --- AG left-mult: q1[d, m'] = sum_m t1[m, d] AGT[m, m'] ---
                q1 = []
                for j in range(DCH):
                    q_ps = psB.tile([128, M], F32, name="mm")
                    for k in range(MCH):
                        nc.tensor.matmul(q_ps, t1[k][:, j * 128:(j + 1) * 128],
                                         agt[:, k, :],
                                         start=(k == 0), stop=(k == MCH - 1))
                    q = q1_pool.tile([128, M], BF16, name="q1")
                    nc.vector.tensor_copy(q, q_ps)
                    q1.append(q)

                # --- Wint right-mult: f3T[d', m] = sum_d Wint[d, d'] q1[d, m] ---
                for j in range(DCH):
                    f_ps = psB.tile([128, M], F32, name="mm")
                    for k in range(DCH):
                        nc.tensor.matmul(f_ps, wint[:, k, j * 128:(j + 1) * 128],
                                         q1[k], start=(k == 0), stop=(k == DCH - 1))
                    f = f3t_pool.tile([128, M], BF16, name="f3t")
                    if cfg["bint_zero"]:
                        nc.vector.tensor_copy(f, f_ps)
                    else:
                        nc.vector.tensor_scalar_add(f, f_ps, bint[:, j:j + 1])
                    f3t[b][j] = f

                # --- W1 + gelu: GT[d', m] = gelu(sum_d W1[d,d'] f3T[d,m] + b1) ---
                gt = []
                for j in range(DCH):
                    g_ps = psB.tile([128, M], F32, name="mm")
                    for k in range(DCH):
                        nc.tensor.matmul(g_ps, w1[:, k, j * 128:(j + 1) * 128],
                                         f3t[b][k], start=(k == 0),
                                         stop=(k == DCH - 1))
                    g = gt_pool.tile([128, M], BF16, name="gt")
                    nc.scalar.activation(out=g, in_=g_ps, func=AF.Gelu_apprx_tanh,
                                         bias=b1c[:, j:j + 1], scale=1.0)
                    gt.append(g)

                # --- W2 + residual: f4T = f3T + (G@W2)^T (+b2) ---
                for j in range(DCH):
                    r_ps = psB.tile([128, M], F32, name="mm")
                    for k in range(DCH):
                        nc.tensor.matmul(r_ps, w2[:, k, j * 128:(j + 1) * 128],
                                         gt[k], start=(k == 0), stop=(k == DCH - 1))
                    f4 = f4t_pool.tile([128, M], BF16, name="f4t")
                    if cfg["b2_zero"]:
                        nc.vector.tensor_add(f4, r_ps, f3t[b][j])
                    else:
                        nc.vector.scalar_tensor_tensor(
                            out=f4, in0=r_ps, scalar=b2c[:, j:j + 1],
                            in1=f3t[b][j], op0=ALU.add, op1=ALU.add)
                    # stage to DRAM for transpose
                    f4t_dram[b] = f4t_dram[b] if f4t_dram[b] is not None else \
                        dram_pool.tile([D, M], BF16, name="f4t")
                    nc.sync.dma_start(out=f4t_dram[b][j * 128:(j + 1) * 128, :],
                                      in_=f4)

                # --- transpose f4T -> f4 [m, d] via DMA xbar ---
                for k in range(MCH):
                    fn = f4n_pool.tile([128, D], BF16, name="f4n")
                    nc.sync.dma_start_transpose(
                        fn, f4t_dram[b][:, k * 128:(k + 1) * 128])
                    f4n[b][k] = fn

            # ================= phase B: sampling + LN1 + Wfinal + LN2 =========
            z_drams = [dram_pool.tile([N, D], BF16, name="z") for _ in range(BPC)]
            for b in range(BPC):
                xs = []
                sumx = stats.tile([128, NCH], F32, name="sumx")
                sumsqx = stats.tile([128, NCH], F32, name="sumsqx")
                for i in range(NCH):
                    s_ps = psC.tile([128, D], F32, name="row")
                    for k in range(MCH):
                        nc.tensor.matmul(s_ps, sat_s[:, b, k, i * 128:(i + 1) * 128],
                                         f4n[b][k], start=(k == 0),
                                         stop=(k == MCH - 1))
                    x = x_pool.tile([128, D], BF16, name="x")
                    nc.vector.scalar_tensor_tensor(
                        out=x, in0=s_ps, scalar=0.0, in1=emb_s[:, b, i, :],
                        op0=ALU.bypass, op1=ALU.add,
                        accum_out=sumx[:, i:i + 1])
                    scr = scr_pool.tile([128, D], BF16, name="sqscr")
                    nc.scalar.activation(out=scr, in_=x, func=AF.Square,
                                         accum_out=sumsqx[:, i:i + 1])
                    xs.append(x)

                # stats -> mu, rstd  (mu = sumx/D; var = sumsq/D - mu^2)
                mu = stats.tile([128, NCH], F32, name="mu")
                nc.vector.tensor_scalar_mul(mu, sumx, 1.0 / D)
                musq = stats.tile([128, NCH], F32, name="musq")
                nc.vector.tensor_mul(musq, mu, mu)
                var = stats.tile([128, NCH], F32, name="var")
                nc.vector.scalar_tensor_tensor(out=var, in0=sumsqx,
                                               scalar=1.0 / D, in1=musq,
                                               op0=ALU.mult, op1=ALU.subtract)
                sd = stats.tile([128, NCH], F32, name="sd")
                nc.scalar.activation(out=sd, in_=var, func=AF.Sqrt,
                                     bias=eps_t, scale=1.0)
                rs = stats.tile([128, NCH], F32, name="rs")
                nc.vector.reciprocal(rs, sd)

                # z = (x - mu) * rs -> bf16 -> DRAM (for transpose)
                for i in range(NCH):
                    z = z_pool.tile([128, D], BF16, name="z")
                    nc.vector.tensor_scalar(out=z, in0=xs[i],
                                            scalar1=mu[:, i:i + 1],
                                            scalar2=rs[:, i:i + 1],
                                            op0=ALU.subtract, op1=ALU.mult)
                    z_dram = z_drams[b]
                    nc.sync.dma_start(out=z_dram[i * 128:(i + 1) * 128, :], in_=z)

                # zT tiles [d, n] via DMA transpose
                zt = []
                for j in range(DCH):
                    t = zt_pool.tile([128, N], BF16, name="zt")
                    nc.sync.dma_start_transpose(
                        t, z_drams[b][:, j * 128:(j + 1) * 128])
                    zt.append(t)

                # y = z @ Wfinal (+bfinal) ; LN2 stats
                ys = []
                sumy = stats.tile([128, NCH], F32, name="sumy")
                sumsqy = stats.tile([128, NCH], F32, name="sumsqy")
                for i in range(NCH):
                    y_ps = psC.tile([128, D], F32, name="row")
                    for k in range(DCH):
                        nc.tensor.matmul(y_ps, zt[k][:, i * 128:(i + 1) * 128],
                                         wfin[:, k, :], start=(k == 0),
                                         stop=(k == DCH - 1))
                    y = y_pool.tile([128, D], F32, name="y")
                    if cfg["bfin_zero"]:
                        nc.vector.tensor_scalar(out=y, in0=y_ps, scalar1=1.0,
                                                scalar2=0.0, op0=ALU.mult,
                                                op1=ALU.add,
                                                accum_out=sumy[:, i:i + 1])
                    else:
                        nc.vector.scalar_tensor_tensor(
                            out=y, in0=y_ps, scalar=0.0, in1=bfinb,
                            op0=ALU.bypass, op1=ALU.add,
                            accum_out=sumy[:, i:i + 1])
                    scr = scr_pool.tile([128, D], BF16, name="sqscr2")
                    nc.scalar.activation(out=scr, in_=y, func=AF.Square,
                                         accum_out=sumsqy[:, i:i + 1])
                    ys.append(y)

                mu2 = stats.tile([128, NCH], F32, name="mu2")
                nc.vector.tensor_scalar_mul(mu2, sumy, 1.0 / D)
                musq2 = stats.tile([128, NCH], F32, name="musq2")
                nc.vector.tensor_mul(musq2, mu2, mu2)
                var2 = stats.tile([128, NCH], F32, name="var2")
                nc.vector.scalar_tensor_tensor(out=var2, in0=sumsqy,
                                               scalar=1.0 / D, in1=musq2,
                                               op0=ALU.mult, op1=ALU.subtract)
                sd2 = stats.tile([128, NCH], F32, name="sd2")
                nc.scalar.activation(out=sd2, in_=var2, func=AF.Sqrt,
                                     bias=eps_t, scale=1.0)
                rs2 = stats.tile([128, NCH], F32, name="rs2")
                nc.vector.reciprocal(rs2, sd2)

                for i in range(NCH):
                    nc.vector.tensor_scalar(out=ys[i], in0=ys[i],
                                            scalar1=mu2[:, i:i + 1],
                                            scalar2=rs2[:, i:i + 1],
                                            op0=ALU.subtract, op1=ALU.mult)
                    if not cfg["ln2_identity"]:
                        nc.vector.tensor_mul(ys[i], ys[i], g2b)
                        nc.vector.tensor_add(ys[i], ys[i], b2b)
                    nc.sync.dma_start(out=out_d[b, i], in_=ys[i])

    nc.compile()
    return nc


_PROGRAM_CACHE = {}


def _get_program(cfg):
    key = tuple(sorted(cfg.items()))
    if key not in _PROGRAM_CACHE:
        _PROGRAM_CACHE[key] = _build_program(cfg)
    return _PROGRAM_CACHE[key]


def _host_prep(inputs):
    """Fold parameters on host; build per-core input maps."""
    f32 = np.float32
    emb = np.asarray(inputs["embeddings"], f32)          # [B, N, D]
    pos = np.asarray(inputs["positions"], f32)[..., 0]   # [B, N]
    sigma = float(np.asarray(inputs["sigma"]))
    alpha = np.asarray(inputs["alpha"], np.float64)
    w_int = np.asarray(inputs["w_int"], np.float64)
    b_int = np.asarray(inputs["b_int"], f32)
    diff_coef = np.asarray(inputs["diff_coef"], np.float64)
    W1 = np.asarray(inputs["W1"], f32)
    b1 = np.asarray(inputs["b1"], f32)
    W2 = np.asarray(inputs["W2"], f32)
    b2 = np.asarray(inputs["b2"], f32)
    evo_coef = np.asarray(inputs["evo_coef"], np.float64)
    ln1_g = np.asarray(inputs["ln1_g"], np.float64)
    ln1_b = np.asarray(inputs["ln1_b"], np.float64)
    Wout = np.asarray(inputs["Wout"], np.float64)
    bout = np.asarray(inputs["bout"], np.float64)
    ln2_g = np.asarray(inputs["ln2_g"], f32)
    ln2_b = np.asarray(inputs["ln2_b"], f32)

    assert np.allclose(diff_coef, diff_coef[0]) and np.allclose(evo_coef, evo_coef[0]), \
        "kernel assumes uniform per-channel diffusion coefficients"

    grid = np.linspace(0.0, 1.0, M)
    L = _laplacian(M)
    A3_1 = np.linalg.matrix_power(np.eye(M) + DT * float(diff_coef[0]) * L, PDE_STEPS)
    A3_2 = np.linalg.matrix_power(np.eye(M) + DT * float(evo_coef[0]) * L, PDE_STEPS)

    G = np.exp(-grid ** 2 / (2.0 * sigma ** 2))          # [M]
    AG = A3_1 @ np.diag(G)                               # [M, M]
    T = np.kron(alpha.T, np.eye(DH))
    Wint_eff = np.eye(D) + T @ w_int                     # [D, D]
    Wfinal = np.diag(ln1_g) @ (Wout + np.eye(D))         # [D, D]
    bfinal = ln1_b @ (Wout + np.eye(D)) + bout           # [D]

    cfg = {
        "bint_zero": bool(np.all(b_int == 0)),
        "b2_zero": bool(np.all(b2 == 0)),
        "bfin_zero": bool(np.all(np.abs(bfinal) < 1e-30)),
        "ln2_identity": bool(np.all(ln2_g == 1) and np.all(ln2_b == 0)),
    }

    u = np.clip(pos, 0.0, 1.0) * (M - 1)                 # [B, N]
    bf16 = ml_dtypes.bfloat16
    agt_h = np.ascontiguousarray(AG.T.reshape(MCH, 128, M)).astype(bf16)
    wint_h = np.ascontiguousarray(Wint_eff.reshape(DCH, 128, D)).astype(bf16)
    w1_h = np.ascontiguousarray(W1.reshape(DCH, 128, D)).astype(bf16)
    w2_h = np.ascontiguousarray(W2.reshape(DCH, 128, D)).astype(bf16)
    wfin_h = np.ascontiguousarray(Wfinal.reshape(DCH, 128, D)).astype(bf16)
    grid_h = np.ascontiguousarray(np.broadcast_to(grid.astype(f32), (128, M)))
    bint_h = np.ascontiguousarray(b_int.reshape(DCH, 128).T).astype(f32)
    b1_h = np.ascontiguousarray(b1.reshape(DCH, 128).T).astype(f32)
    b2_h = np.ascontiguousarray(b2.reshape(DCH, 128).T).astype(f32)
    bfin_h = np.ascontiguousarray(np.broadcast_to(bfinal.astype(f32), (128, D)))
    g2_h = np.ascontiguousarray(np.broadcast_to(ln2_g, (128, D)))
    b2b_h = np.ascontiguousarray(np.broadcast_to(ln2_b, (128, D)))

    in_maps = []
    for c in range(NCORES):
        bs = slice(c * BPC, (c + 1) * BPC)
        emb_c = np.ascontiguousarray(emb[bs].reshape(BPC, NCH, 128, D)).astype(bf16)
        pos_c = pos[bs]                                   # [BPC, N]
        psc_c = (pos_c / sigma ** 2).reshape(BPC, NCH, 128).transpose(0, 2, 1)
        pbi_c = (-pos_c ** 2 / (2 * sigma ** 2)).reshape(BPC, NCH, 128).transpose(0, 2, 1)
        u_c = u[bs]                                       # [BPC, N]
        S = np.maximum(0.0, 1.0 - np.abs(u_c[:, :, None] - np.arange(M)[None, None, :]))
        SA = S @ A3_2                                     # [BPC, N, M]
        sat_c = np.ascontiguousarray(SA.transpose(0, 2, 1).reshape(
            BPC, MCH, 128, N)).astype(bf16)
        in_maps.append({
            "emb": emb_c,
            "psc": np.ascontiguousarray(psc_c, f32),
            "pbi": np.ascontiguousarray(pbi_c, f32),
            "gridb": grid_h,
            "agt": agt_h,
            "wint": wint_h,
            "w1": w1_h,
            "w2": w2_h,
            "wfin": wfin_h,
            "sat": sat_c,
            "bint": bint_h,
            "b1c": b1_h,
            "b2c": b2_h,
            "bfinb": bfin_h,
            "g2b": g2_h,
            "b2b": b2b_h,
        })
    return cfg, in_maps


def kernel(**inputs) -> np.ndarray:
    cfg, in_maps = _host_prep(inputs)
    nc = _get_program(cfg)
    res = run_bass_kernel_spmd(nc, in_maps, core_ids=list(range(NCORES)))
    out = np.empty((B, N, D), np.float32)
    for c in range(NCORES):
        o = res.results[c]["out"]                         # [BPC, NCH, 128, D]
        out[c * BPC:(c + 1) * BPC] = o.reshape(BPC, N, D)
    return out


# revision 22
# speedup vs baseline: 1.2028x; 1.2028x over previous
"""Trainium2 Bass kernel for nn_EnhancedTFNLayer (RBF field projection +
head interference + diffusion + gated MLP + diffusion + resampling + LN/proj/LN).

Data-parallel over batch: B=16 split as 2 batches per core across 8 cores.
All heavy math runs on device; small parameter matrices are folded on host:
  - stage-2 head mixing:  Wint_eff = I + kron(alpha.T, I_DH) @ w_int
  - diffusion (uniform per-channel coef): left-multiply by A3 = (I + dt*c*L)^3
  - RBF row scale G folded into AG = A3_1 @ diag(G)
  - sampling + 2nd diffusion: SA = S @ A3_2  (hat-function interp matrix S)
  - LN1 gamma/beta + both residuals around Wout: Wfinal = diag(g1) @ (Wout + I)
"""
import sys

sys.path.insert(0, "/opt/trn_rl_repo")

import numpy as np
import ml_dtypes

import concourse.bass as bass
import concourse.tile as tile
from concourse.masks import make_identity
from concourse import mybir, bacc
from concourse.bass_utils import run_bass_kernel_spmd

B, N, D, M, H = 16, 2048, 512, 512, 8
DH = D // H
PDE_STEPS = 3
DT = 0.01
NCORES = 8
BPC = B // NCORES  # batches per core
NCH = N // 128     # 16 token chunks
DCH = D // 128     # 4
MCH = M // 128     # 4

F32 = mybir.dt.float32
BF16 = mybir.dt.bfloat16
AF = mybir.ActivationFunctionType
ALU = mybir.AluOpType


def _laplacian(m):
    L = np.zeros((m, m), np.float64)
    for i in range(m):
        L[i, i] -= 2.0
        L[i, max(i - 1, 0)] += 1.0
        L[i, min(i + 1, m - 1)] += 1.0
    return L


def _build_program(cfg):
    """cfg: dict of identity-flags controlling optional passes."""
    nc = bacc.Bacc("TRN2", target_bir_lowering=False, debug=False,
                   enable_asserts=False, num_devices=NCORES)

    # ---- external inputs (per core) ----
    emb_d = nc.dram_tensor("emb", (BPC, NCH, 128, D), BF16, kind="ExternalInput")
    # packed: grid(M) + psc(BPC*NCH) + pbi(BPC*NCH) + bint(DCH) + b1(DCH)
    #         + b2(DCH) [+ bfin(D) if used] [+ g2(D) + b2b(D) if used]
    CPACK = (M + 2 * BPC * NCH + 3 * DCH
             + (0 if cfg["bfin_zero"] else D)
             + (0 if cfg["ln2_identity"] else 2 * D))
    cpack_d = nc.dram_tensor("cpack", (128, CPACK), F32, kind="ExternalInput")
    wpack_d = nc.dram_tensor("wpack", (5, DCH, 128, D), BF16, kind="ExternalInput")
    sat_d = nc.dram_tensor("sat", (BPC, MCH, 128, N), BF16, kind="ExternalInput")

    out_d = nc.dram_tensor("out", (BPC, NCH, 128, D), F32, kind="ExternalOutput")


    from contextlib import ExitStack
    with tile.TileContext(nc) as tc, ExitStack() as ctx:
        consts = ctx.enter_context(tc.tile_pool(name="consts", bufs=1))
        ep_pool = ctx.enter_context(tc.tile_pool(name="ep", bufs=4))
        t1_pool = ctx.enter_context(tc.tile_pool(name="t1p", bufs=MCH + 1))
        q1_pool = ctx.enter_context(tc.tile_pool(name="q1p", bufs=DCH + 1))
        f3t_pool = ctx.enter_context(tc.tile_pool(name="f3p", bufs=2 * DCH + 1))
        gt_pool = ctx.enter_context(tc.tile_pool(name="gtp", bufs=DCH + 1))
        f4t_pool = ctx.enter_context(tc.tile_pool(name="f4p", bufs=DCH + 1))
        f4n_pool = ctx.enter_context(tc.tile_pool(name="f4n", bufs=BPC * MCH + 1))
        dram_pool = ctx.enter_context(tc.tile_pool(name="dram", bufs=2, space="DRAM"))
        x_pool = ctx.enter_context(tc.tile_pool(name="xp", bufs=NCH + 1))
        y_pool = ctx.enter_context(tc.tile_pool(name="yp", bufs=1))
        z_pool = ctx.enter_context(tc.tile_pool(name="zp", bufs=2))
        zt_pool = ctx.enter_context(tc.tile_pool(name="ztp", bufs=DCH))
        scr_pool = ctx.enter_context(tc.tile_pool(name="scr", bufs=1))
        stats = ctx.enter_context(tc.tile_pool(name="st", bufs=2))
        psA = ctx.enter_context(tc.tile_pool(name="psA", bufs=MCH, space="PSUM"))
        psB = ctx.enter_context(tc.tile_pool(name="psB", bufs=2, space="PSUM"))
        psC = ctx.enter_context(tc.tile_pool(name="psC", bufs=2, space="PSUM"))
        if True:
            # ---- load constants ----
            cpack = consts.tile([128, CPACK], F32)
            nc.sync.dma_start(out=cpack, in_=cpack_d[:, :])
            o = 0
            grid_b = cpack[:, o:o + M]; o += M
            psc = cpack[:, o:o + BPC * NCH].rearrange("p (b c) -> p b c", b=BPC); o += BPC * NCH
            pbi = cpack[:, o:o + BPC * NCH].rearrange("p (b c) -> p b c", b=BPC); o += BPC * NCH
            bint = cpack[:, o:o + DCH]; o += DCH
            b1c = cpack[:, o:o + DCH]; o += DCH
            b2c = cpack[:, o:o + DCH]; o += DCH
            if not cfg["bfin_zero"]:
                bfinb = cpack[:, o:o + D]; o += D
            if not cfg["ln2_identity"]:
                g2b = cpack[:, o:o + D]; o += D
                b2b = cpack[:, o:o + D]; o += D
            wpack = consts.tile([128, 5, DCH, D], BF16)
            nc.sync.dma_start(out=wpack, in_=wpack_d.rearrange("w c p d -> p w c d"))
            agt = wpack[:, 0]
            wint = wpack[:, 1]
            w1 = wpack[:, 2]
            w2 = wpack[:, 3]
            wfin = wpack[:, 4]
            eps_t = consts.tile([128, 1], F32)
            nc.vector.memset(eps_t, 1e-5)
            ident = consts.tile([128, 128], BF16)
            make_identity(nc, ident)

            emb_s = consts.tile([128, BPC, NCH, D], BF16)
            for b in range(BPC):
                nc.sync.dma_start(out=emb_s[:, b],
                                  in_=emb_d[b].rearrange("c p d -> p c d"))
            sat_s = consts.tile([128, BPC, MCH, N], BF16)
            for b in range(BPC):
                nc.sync.dma_start(out=sat_s[:, b],
                                  in_=sat_d[b].rearrange("c p n -> p c n"))

            # per-batch persistent intermediates
            f3t = [[None] * DCH for _ in range(BPC)]
            f4n = [[None] * MCH for _ in range(BPC)]

            # ================= phase A: EP + RBF + AG + Wint + W1 + W2 ========
            for b in range(BPC):
                # --- EP build + RBF matmul (t1[m,d] = sum_n EP[n,m] emb[n,d]) ---
                t1_ps = [psA.tile([128, D], F32, name="t1ps") for _ in range(MCH)]
                for i in range(NCH):
                    ep = ep_pool.tile([128, M], BF16, name="ep")
                    nc.scalar.activation(out=ep, in_=grid_b, func=AF.Exp,
                                         scale=psc[:, b, i:i + 1],
                                         bias=pbi[:, b, i:i + 1])
                    for j in range(MCH):
                        nc.tensor.matmul(t1_ps[j], ep[:, j * 128:(j + 1) * 128],
                                         emb_s[:, b, i, :],
                                         start=(i == 0), stop=(i == NCH - 1))
                t1 = []
                for j in range(MCH):
                    t = t1_pool.tile([128, D], BF16, name="t1")
                    nc.vector.tensor_copy(t, t1_ps[j])
                    t1.append(t)

                # --- AG left-mult: q1[d, m'] = sum_m t1[m, d] AGT[m, m'] ---
                q1 = []
                for j in range(DCH):
                    q_ps = psB.tile([128, M], F32, name="mm")
                    for k in range(MCH):
                        nc.tensor.matmul(q_ps, t1[k][:, j * 128:(j + 1) * 128],
                                         agt[:, k, :],
                                         start=(k == 0), stop=(k == MCH - 1))
                    q = q1_pool.tile([128, M], BF16, name="q1")
                    nc.vector.tensor_copy(q, q_ps)
                    q1.append(q)

                # --- Wint right-mult: f3T[d', m] = sum_d Wint[d, d'] q1[d, m] ---
                for j in range(DCH):
                    f_ps = psB.tile([128, M], F32, name="mm")
                    for k in range(DCH):
                        nc.tensor.matmul(f_ps, wint[:, k, j * 128:(j + 1) * 128],
                                         q1[k], start=(k == 0), stop=(k == DCH - 1))
                    f = f3t_pool.tile([128, M], BF16, name="f3t")
                    if cfg["bint_zero"]:
                        nc.vector.tensor_copy(f, f_ps)
                    else:
                        nc.vector.tensor_scalar_add(f, f_ps, bint[:, j:j + 1])
                    f3t[b][j] = f

                # --- W1 + gelu: GT[d', m] = gelu(sum_d W1[d,d'] f3T[d,m] + b1) ---
                gt = []
                for j in range(DCH):
                    g_ps = psB.tile([128, M], F32, name="mm")
                    for k in range(DCH):
                        nc.tensor.matmul(g_ps, w1[:, k, j * 128:(j + 1) * 128],
                                         f3t[b][k], start=(k == 0),
                                         stop=(k == DCH - 1))
                    g = gt_pool.tile([128, M], BF16, name="gt")
                    nc.scalar.activation(out=g, in_=g_ps, func=AF.Gelu_apprx_tanh,
                                         bias=b1c[:, j:j + 1], scale=1.0)
                    gt.append(g)

                # --- W2 + residual: f4T = f3T + (G@W2)^T (+b2) ---
                f4ts = []
                for j in range(DCH):
                    r_ps = psB.tile([128, M], F32, name="mm")
                    for k in range(DCH):
                        nc.tensor.matmul(r_ps, w2[:, k, j * 128:(j + 1) * 128],
                                         gt[k], start=(k == 0), stop=(k == DCH - 1))
                    f4 = f4t_pool.tile([128, M], BF16, name="f4t")
                    if cfg["b2_zero"]:
                        nc.vector.tensor_add(f4, r_ps, f3t[b][j])
                    else:
                        nc.vector.scalar_tensor_tensor(
                            out=f4, in0=r_ps, scalar=b2c[:, j:j + 1],
                            in1=f3t[b][j], op0=ALU.add, op1=ALU.add)
                    f4ts.append(f4)

                # --- transpose f4T -> f4 [m, d] via PE transpose ---
                for k in range(MCH):
                    tp_ps = psB.tile([128, M], BF16, name="mm")
                    for j in range(DCH):
                        nc.tensor.transpose(tp_ps[:, j * 128:(j + 1) * 128],
                                            f4ts[j][:, k * 128:(k + 1) * 128],
                                            ident)
                    fn = f4n_pool.tile([128, D], BF16, name="f4n")
                    nc.scalar.copy(fn, tp_ps)
                    f4n[b][k] = fn

            # ================= phase B: sampling + LN1 + Wfinal + LN2 =========
            for b in range(BPC):
                xs = []
                sumx = stats.tile([128, NCH], F32, name="sumx")
                sumsqx = stats.tile([128, NCH], F32, name="sumsqx")
                for i in range(NCH):
                    s_ps = psC.tile([128, D], F32, name="row")
                    for k in range(MCH):
                        nc.tensor.matmul(s_ps, sat_s[:, b, k, i * 128:(i + 1) * 128],
                                         f4n[b][k], start=(k == 0),
                                         stop=(k == MCH - 1))
                    x = x_pool.tile([128, D], BF16, name="x")
                    nc.vector.scalar_tensor_tensor(
                        out=x, in0=s_ps, scalar=0.0, in1=emb_s[:, b, i, :],
                        op0=ALU.bypass, op1=ALU.add,
                        accum_out=sumx[:, i:i + 1])
                    scr = scr_pool.tile([128, D], BF16, name="sqscr")
                    nc.scalar.activation(out=scr, in_=x, func=AF.Square,
                                         accum_out=sumsqx[:, i:i + 1])
                    xs.append(x)

                # stats -> mu, rstd  (mu = sumx/D; var = sumsq/D - mu^2)
                mu = stats.tile([128, NCH], F32, name="mu")
                nc.vector.tensor_scalar_mul(mu, sumx, 1.0 / D)
                musq = stats.tile([128, NCH], F32, name="musq")
                nc.vector.tensor_mul(musq, mu, mu)
                var = stats.tile([128, NCH], F32, name="var")
                nc.vector.scalar_tensor_tensor(out=var, in0=sumsqx,
                                               scalar=1.0 / D, in1=musq,
                                               op0=ALU.mult, op1=ALU.subtract)
                sd = stats.tile([128, NCH], F32, name="sd")
                nc.scalar.activation(out=sd, in_=var, func=AF.Sqrt,
                                     bias=eps_t, scale=1.0)
                rs = stats.tile([128, NCH], F32, name="rs")
                nc.vector.reciprocal(rs, sd)

                # z = (x - mu) * rs -> bf16, then PE-transpose into zt [d, n]
                zt = [zt_pool.tile([128, N], BF16, name="zt")
                      for _ in range(DCH)]
                for g in range(NCH // 4):
                    zb = z_pool.tile([128, 4, D], BF16, name="zb")
                    for s in range(4):
                        i = g * 4 + s
                        nc.vector.tensor_scalar(out=zb[:, s, :], in0=xs[i],
                                                scalar1=mu[:, i:i + 1],
                                                scalar2=rs[:, i:i + 1],
                                                op0=ALU.subtract, op1=ALU.mult)
                    for j in range(DCH):
                        tp_ps = psB.tile([128, M], BF16, name="mm")
                        for s in range(4):
                            nc.tensor.transpose(
                                tp_ps[:, s * 128:(s + 1) * 128],
                                zb[:, s, j * 128:(j + 1) * 128], ident)
                        nc.scalar.copy(zt[j][:, g * 512:(g + 1) * 512], tp_ps)

                # y = z @ Wfinal (+bfinal) ; LN2 stats
                ybuf = y_pool.tile([128, NCH, D], F32, name="ybuf")
                ys = []
                sumy = stats.tile([128, NCH], F32, name="sumy")
                sumsqy = stats.tile([128, NCH], F32, name="sumsqy")
                for i in range(NCH):
                    y_ps = psC.tile([128, D], F32, name="row")
                    for k in range(DCH):
                        nc.tensor.matmul(y_ps, zt[k][:, i * 128:(i + 1) * 128],
                                         wfin[:, k, :], start=(k == 0),
                                         stop=(k == DCH - 1))
                    y = ybuf[:, i, :]
                    if cfg["bfin_zero"]:
                        nc.vector.tensor_scalar(out=y, in0=y_ps, scalar1=1.0,
                                                scalar2=0.0, op0=ALU.mult,
                                                op1=ALU.add,
                                                accum_out=sumy[:, i:i + 1])
                    else:
                        nc.vector.scalar_tensor_tensor(
                            out=y, in0=y_ps, scalar=0.0, in1=bfinb,
                            op0=ALU.bypass, op1=ALU.add,
                            accum_out=sumy[:, i:i + 1])
                    scr = scr_pool.tile([128, D], BF16, name="sqscr2")
                    nc.scalar.activation(out=scr, in_=y, func=AF.Square,
                                         accum_out=sumsqy[:, i:i + 1])
                    ys.append(y)

                mu2 = stats.tile([128, NCH], F32, name="mu2")
                nc.vector.tensor_scalar_mul(mu2, sumy, 1.0 / D)
                musq2 = stats.tile([128, NCH], F32, name="musq2")
                nc.vector.tensor_mul(musq2, mu2, mu2)
                var2 = stats.tile([128, NCH], F32, name="var2")
                nc.vector.scalar_tensor_tensor(out=var2, in0=sumsqy,
                                               scalar=1.0 / D, in1=musq2,
                                               op0=ALU.mult, op1=ALU.subtract)
                sd2 = stats.tile([128, NCH], F32, name="sd2")
                nc.scalar.activation(out=sd2, in_=var2, func=AF.Sqrt,
                                     bias=eps_t, scale=1.0)
                rs2 = stats.tile([128, NCH], F32, name="rs2")
                nc.vector.reciprocal(rs2, sd2)

                for i in range(NCH):
                    nc.vector.tensor_scalar(out=ys[i], in0=ys[i],
                                            scalar1=mu2[:, i:i + 1],
                                            scalar2=rs2[:, i:i + 1],
                                            op0=ALU.subtract, op1=ALU.mult)
                    if not cfg["ln2_identity"]:
                        nc.vector.tensor_mul(ys[i], ys[i], g2b)
                        nc.vector.tensor_add(ys[i], ys[i], b2b)
                for i in range(NCH):
                    nc.sync.dma_start(out=out_d[b, i], in_=ys[i])

    nc.compile()
    return nc


_PROGRAM_CACHE = {}


def _get_program(cfg):
    key = tuple(sorted(cfg.items()))
    if key not in _PROGRAM_CACHE:
        _PROGRAM_CACHE[key] = _build_program(cfg)
    return _PROGRAM_CACHE[key]


def _host_prep(inputs):
    """Fold parameters on host; build per-core input maps."""
    f32 = np.float32
    emb = np.asarray(inputs["embeddings"], f32)          # [B, N, D]
    pos = np.asarray(inputs["positions"], f32)[..., 0]   # [B, N]
    sigma = float(np.asarray(inputs["sigma"]))
    alpha = np.asarray(inputs["alpha"], np.float64)
    w_int = np.asarray(inputs["w_int"], np.float64)
    b_int = np.asarray(inputs["b_int"], f32)
    diff_coef = np.asarray(inputs["diff_coef"], np.float64)
    W1 = np.asarray(inputs["W1"], f32)
    b1 = np.asarray(inputs["b1"], f32)
    W2 = np.asarray(inputs["W2"], f32)
    b2 = np.asarray(inputs["b2"], f32)
    evo_coef = np.asarray(inputs["evo_coef"], np.float64)
    ln1_g = np.asarray(inputs["ln1_g"], np.float64)
    ln1_b = np.asarray(inputs["ln1_b"], np.float64)
    Wout = np.asarray(inputs["Wout"], np.float64)
    bout = np.asarray(inputs["bout"], np.float64)
    ln2_g = np.asarray(inputs["ln2_g"], f32)
    ln2_b = np.asarray(inputs["ln2_b"], f32)

    assert np.allclose(diff_coef, diff_coef[0]) and np.allclose(evo_coef, evo_coef[0]), \
        "kernel assumes uniform per-channel diffusion coefficients"

    grid = np.linspace(0.0, 1.0, M)
    L = _laplacian(M)
    A3_1 = np.linalg.matrix_power(np.eye(M) + DT * float(diff_coef[0]) * L, PDE_STEPS)
    A3_2 = np.linalg.matrix_power(np.eye(M) + DT * float(evo_coef[0]) * L, PDE_STEPS)

    G = np.exp(-grid ** 2 / (2.0 * sigma ** 2))          # [M]
    AG = A3_1 @ np.diag(G)                               # [M, M]
    T = np.kron(alpha.T, np.eye(DH))
    Wint_eff = np.eye(D) + T @ w_int                     # [D, D]
    Wfinal = np.diag(ln1_g) @ (Wout + np.eye(D))         # [D, D]
    bfinal = ln1_b @ (Wout + np.eye(D)) + bout           # [D]

    cfg = {
        "bint_zero": bool(np.all(b_int == 0)),
        "b2_zero": bool(np.all(b2 == 0)),
        "bfin_zero": bool(np.all(np.abs(bfinal) < 1e-30)),
        "ln2_identity": bool(np.all(ln2_g == 1) and np.all(ln2_b == 0)),
    }

    u = np.clip(pos, 0.0, 1.0) * (M - 1)                 # [B, N]
    bf16 = ml_dtypes.bfloat16
    wpack_h = np.ascontiguousarray(np.stack([
        AG.T.reshape(DCH, 128, D),
        Wint_eff.reshape(DCH, 128, D),
        W1.astype(np.float64).reshape(DCH, 128, D),
        W2.astype(np.float64).reshape(DCH, 128, D),
        Wfinal.reshape(DCH, 128, D),
    ])).astype(bf16)

    bint_h = b_int.reshape(DCH, 128).T.astype(f32)       # [128, DCH]
    b1_h = b1.reshape(DCH, 128).T.astype(f32)
    b2_h = b2.reshape(DCH, 128).T.astype(f32)
    grid_h = np.broadcast_to(grid.astype(f32), (128, M))
    bfin_h = np.broadcast_to(bfinal.astype(f32), (128, D))
    g2_h = np.broadcast_to(ln2_g, (128, D))
    b2b_h = np.broadcast_to(ln2_b, (128, D))

    in_maps = []
    for c in range(NCORES):
        bs = slice(c * BPC, (c + 1) * BPC)
        emb_c = np.ascontiguousarray(emb[bs].reshape(BPC, NCH, 128, D)).astype(bf16)
        pos_c = pos[bs]                                   # [BPC, N]
        psc_c = (pos_c / sigma ** 2).reshape(BPC, NCH, 128).transpose(0, 2, 1)
        pbi_c = (-pos_c ** 2 / (2 * sigma ** 2)).reshape(BPC, NCH, 128).transpose(0, 2, 1)
        parts = [grid_h,
                 psc_c.transpose(1, 0, 2).reshape(128, BPC * NCH),
                 pbi_c.transpose(1, 0, 2).reshape(128, BPC * NCH),
                 bint_h, b1_h, b2_h]
        if not cfg["bfin_zero"]:
            parts.append(bfin_h)
        if not cfg["ln2_identity"]:
            parts += [g2_h, b2b_h]
        cpack_c = np.concatenate(parts, axis=1).astype(f32)
        u_c = u[bs]                                       # [BPC, N]
        # SA = S @ A3_2 where S has 2 nonzeros/row -> weighted gather of A3 rows
        i0 = np.clip(np.floor(u_c), 0, M - 2).astype(np.int64)
        w = (u_c - i0)[..., None].astype(np.float32)      # [BPC, N, 1]
        A3f = A3_2.astype(np.float32)
        SA = (1.0 - w) * A3f[i0] + w * A3f[i0 + 1]        # [BPC, N, M]
        sat_c = np.ascontiguousarray(SA.transpose(0, 2, 1).reshape(
            BPC, MCH, 128, N)).astype(bf16)
        in_maps.append({
            "emb": emb_c,
            "cpack": np.ascontiguousarray(cpack_c),
            "wpack": wpack_h,
            "sat": sat_c,
        })
    return cfg, in_maps


def kernel(**inputs) -> np.ndarray:
    cfg, in_maps = _host_prep(inputs)
    nc = _get_program(cfg)
    res = run_bass_kernel_spmd(nc, in_maps, core_ids=list(range(NCORES)))
    out = np.empty((B, N, D), np.float32)
    for c in range(NCORES):
        o = res.results[c]["out"]                         # [BPC, NCH, 128, D]
        out[c * BPC:(c + 1) * BPC] = o.reshape(BPC, N, D)
    return out
